# revision 3
# baseline (speedup 1.0000x reference)
"""Trainium2 Bass kernel for the spike-decoder GNN message-passing module.

Math (per batch b, output time tau in [0, T-2], variable v):
  out[b,tau,v] = bias[v]
               + sum_{i,k} w[v,i,k] * x[b,i,tau+k-(K-2)]          (static conv)
               + sum_{e: recv[e]=v} sum_k dw[e,b,tau,k] * x[b,send[e],tau+k-(K-2)]
with w = conv_weight masked at w[i,i,K-1] = 0, x = spikes[...,0] transposed to
[b, nvar, t], and out-of-range x treated as zero.

Sharding: 8 cores = (b in 0..3) x (time half h in 0..1). Each core computes a
1024-wide tau window ([0,1024) or [1023,2047) — one overlapping column keeps
shapes uniform for SPMD). dyn_weights is the only big tensor; it streams as
bf16 (the 2e-2 tolerance leaves ~10x headroom: measured 1.7e-3) which halves
the memory-bound DMA to ~17 MB/core.

On-core algorithm (bf16 operands, fp32 PSUM accumulation — exact except for
the host-side bf16 rounding of dw/w, since x is 0/1):
  - xg[e,:] = x[send[e],:] gathered via one-hot matmul on PE (doubles as the
    HAM clock warmup); ScalarE writes TWO copies into one tile (cols 0:1040
    even, cols 1040:2080 shifted by one column) so every DVE window AP starts
    4B-aligned.
  - products P[e,(par,k,tau)] = dw_tile * sliding-window(xg) on DVE. dw
    arrives parity-major ([even ks | odd ks]); ONE 4D-AP tensor_tensor per
    tile covers both parities (par dim strides 1040 in xg, HK in dw/P) with
    all operands 2-byte, stride-1, 4B-aligned -> DVE 2x_1p (2 elem/cyc/lane).
  - k-reduction + recv-scatter + transpose folded into PE: per k, a bf16
    matmul with stationary one-hot recv matrix and moving operand = product
    chunk P[:, mC..mC+C], accumulating into PSUM[v, tau].
  - static conv: 16 bf16 matmuls with stationary wT_k and shifted xpad slices
  - bias: folded into the ScalarE PSUM->SBUF copy (Identity + bias AP, fp32)
Queues: dw stream alone on the sync/HWDGE queue (descriptor-gen for the big
stream starts immediately); consts + output stores on the gpsimd/SWDGE queue.
The last tile is DMA'd and multiplied in quarters to shrink the serial tail.
Output is [v, tau] per core; host transposes while assembling the result.
"""

import numpy as np

B, T, NVAR, K, E = 4, 2048, 128, 16, 512
TAU = T - 1            # 2047
L = 1024               # per-core tau window
NC_COUNT = 8
W_XPAD = L + K         # 1040 (1039 used; even so bf16 tiles stay 4B-aligned)
ETILES = E // 128      # 4
CHUNK = 512            # tau chunk per PSUM bank
NCHUNK = L // CHUNK    # 2
KH = K // 2            # 8 ks per parity
HK = CHUNK * KH        # 4096 product columns per parity half
QW = HK // 2           # 2048 columns per tail quarter

_PROGRAM = None


def _build_program():
    import concourse.bass as bass
    import concourse.bacc as bacc
    import concourse.mybir as mybir
    import concourse.tile as tile

    f32 = mybir.dt.float32
    bf16 = mybir.dt.bfloat16
    # Bacc (not plain Bass): its compile pipeline runs generate_event_semaphores,
    # which splits multi-semaphore waits — a raw Matmult supports only one
    # sync-wait slot and walrus rejects more ("Too many sync wait commands").
    nc = bacc.Bacc()

    xpad_d = nc.declare_dram_parameter("xpad", [NVAR, W_XPAD], bf16, isOutput=False)
    dw_d = nc.declare_dram_parameter("dw", [NCHUNK * E, CHUNK * K], bf16, isOutput=False)
    ssend_d = nc.declare_dram_parameter("ssend", [NVAR, E], bf16, isOutput=False)
    wt_d = nc.declare_dram_parameter("wt", [NVAR, K * NVAR], bf16, isOutput=False)
    recv_d = nc.declare_dram_parameter("recvT", [128, ETILES * NVAR], bf16, isOutput=False)
    bias_d = nc.declare_dram_parameter("biasv", [NVAR, 1], f32, isOutput=False)
    y_d = nc.declare_dram_parameter("yT", [NVAR, L], f32, isOutput=True)

    with tile.TileContext(nc) as tc:
        with (
            tc.tile_pool(name="consts", bufs=1) as consts,
            tc.tile_pool(name="xgp", bufs=1) as xgp,
            tc.tile_pool(name="gpsum", bufs=2, space=bass.MemorySpace.PSUM) as gpsum,
            tc.tile_pool(name="dwp", bufs=4) as dwp,
            tc.tile_pool(name="prodp", bufs=3) as prodp,
            tc.tile_pool(name="opsum", bufs=2, space=bass.MemorySpace.PSUM) as opsum,
            tc.tile_pool(name="resp", bufs=2) as resp,
        ):
            NT = NCHUNK * ETILES  # 8 dw tiles

            # consts ride the gpsimd/SWDGE queue so the sync/HWDGE queue's
            # descriptor generator belongs to the dw stream from t=0.
            xpad = consts.tile([NVAR, W_XPAD], bf16)
            nc.gpsimd.dma_start(xpad[:], xpad_d[:])
            ssend = consts.tile([NVAR, E], bf16)
            nc.gpsimd.dma_start(ssend[:], ssend_d[:])
            wt = consts.tile([NVAR, K * NVAR], bf16)
            nc.gpsimd.dma_start(wt[:], wt_d[:])
            recvT = consts.tile([128, ETILES * NVAR], bf16)
            nc.gpsimd.dma_start(recvT[:], recv_d[:])
            biasv = consts.tile([NVAR, 1], f32)
            nc.gpsimd.dma_start(biasv[:], bias_d[:])

            dwt_tiles = []
            for ti in range(NT):
                dwt = dwp.tile([128, CHUNK * K], bf16, name="dwt", tag="dwt")
                dwt_tiles.append(dwt)
            for ti in range(NT):
                h2, et = divmod(ti, ETILES)
                r0 = h2 * E + et * 128
                if ti == NT - 1:
                    # tail tile streams in quarters so its multiply+reduce
                    # overlaps the final DMA packets
                    for q in range(4):
                        nc.sync.dma_start(
                            dwt_tiles[ti][:, q * QW:(q + 1) * QW],
                            dw_d[r0:r0 + 128, q * QW:(q + 1) * QW],
                        )
                else:
                    nc.sync.dma_start(dwt_tiles[ti][:], dw_d[r0:r0 + 128, :])

            # Gather sender rows (also the PE clock warmup):
            # xgc[et][p, j]      = xpad[send[et*128+p], j]          (cols 0:1040)
            # xgc[et][p, 1040+j] = xpad[send[et*128+p], j+1]        (odd-k copy)
            xgc = []
            for et in range(ETILES):
                xg = xgp.tile([128, 2 * W_XPAD], bf16, name=f"xg{et}", tag=f"xg{et}")
                for j0 in range(0, W_XPAD, CHUNK):
                    jw = min(CHUNK, W_XPAD - j0)
                    gps = gpsum.tile([128, CHUNK], f32, name="gps", tag="gps")
                    nc.tensor.matmul(
                        gps[:, :jw],
                        ssend[:, et * 128:(et + 1) * 128],
                        xpad[:, j0:j0 + jw],
                        start=True, stop=True,
                    )
                    nc.scalar.copy(xg[:, j0:j0 + jw], gps[:, :jw])
                    s0 = 1 if j0 == 0 else 0
                    nc.scalar.copy(
                        xg[:, W_XPAD + j0 - 1 + s0:W_XPAD + j0 + jw - 1],
                        gps[:, s0:jw],
                    )
                xgc.append(xg)

            ops_tiles = []
            for h2 in range(NCHUNK):
                o = opsum.tile([128, CHUNK], f32, name=f"ops{h2}", tag=f"ops{h2}")
                ops_tiles.append(o)

            def static_mm(h2, k, start=False):
                t0 = h2 * CHUNK
                nc.tensor.matmul(
                    ops_tiles[h2][:],
                    wt[:, k * NVAR:(k + 1) * NVAR],
                    xpad[:, t0 + k:t0 + k + CHUNK],
                    start=start, stop=False,
                )

            # chunk-0 static conv up front (PE warmup continues while dw streams)
            for k in range(K):
                static_mm(0, k, start=(k == 0))

            # chunk-1 static matmuls fill PE gaps across the first 7 groups
            fill = [("s", k) for k in range(K)]
            fills_per_group = [3, 3, 2, 2, 2, 2, 2, 0]

            def reduce_mm(h2, et, pt, prow, m, stop):
                rhs = bass.AP(pt.tensor, m * CHUNK, [[prow, 128], [1, CHUNK]])
                nc.tensor.matmul(
                    ops_tiles[h2][:],
                    recvT[:, et * NVAR:(et + 1) * NVAR],
                    rhs,
                    start=False, stop=stop,
                )

            for ti in range(NT):
                h2, et = divmod(ti, ETILES)
                t0 = h2 * CHUNK
                dwt = dwt_tiles[ti]
                drow = dwt.tensor.shape[-1]
                xrow = xgc[et].tensor.shape[-1]
                # dw is parity-major: dwt[e, par*HK + m*CHUNK + tau] holds
                # dw[e, k=2m+par, tau]; window for k is xgc[par*1040 + t0+2m+tau]
                if ti < NT - 1:
                    pt = prodp.tile([128, CHUNK * K], bf16, name="pt", tag="pt")
                    prow = pt.tensor.shape[-1]
                    in0 = bass.AP(dwt.tensor, 0,
                                  [[drow, 128], [HK, 2], [CHUNK, KH], [1, CHUNK]])
                    in1 = bass.AP(xgc[et].tensor, t0,
                                  [[xrow, 128], [W_XPAD, 2], [2, KH], [1, CHUNK]])
                    out4 = bass.AP(pt.tensor, 0,
                                   [[prow, 128], [HK, 2], [CHUNK, KH], [1, CHUNK]])
                    nc.vector.tensor_mul(out4, in0, in1)
                    for m in range(K):
                        reduce_mm(h2, et, pt, prow, m,
                                  stop=(et == ETILES - 1 and m == K - 1))
                else:
                    # tail tile: quarter-granular multiply + reduce
                    for q in range(4):
                        par, hf = divmod(q, 2)
                        ptq = prodp.tile([128, QW], bf16, name="ptq", tag="ptq")
                        prow = ptq.tensor.shape[-1]
                        in0 = bass.AP(dwt.tensor, q * QW,
                                      [[drow, 128], [CHUNK, KH // 2], [1, CHUNK]])
                        in1 = bass.AP(xgc[et].tensor,
                                      par * W_XPAD + t0 + 2 * (hf * (KH // 2)),
                                      [[xrow, 128], [2, KH // 2], [1, CHUNK]])
                        out3 = bass.AP(ptq.tensor, 0,
                                       [[prow, 128], [CHUNK, KH // 2], [1, CHUNK]])
                        nc.vector.tensor_mul(out3, in0, in1)
                        for m in range(KH // 2):
                            reduce_mm(h2, et, ptq, prow, m,
                                      stop=(q == 3 and m == KH // 2 - 1))
                for _ in range(fills_per_group[ti]):
                    _, k = fill.pop(0)
                    static_mm(1, k, start=(k == 0))
                if et == ETILES - 1:
                    res = resp.tile([128, CHUNK], f32, name="res", tag="res")
                    # PSUM -> SBUF copy with the conv bias added (exact fp32)
                    nc.scalar.add(res[:], ops_tiles[h2][:], biasv[:, 0:1])
                    nc.gpsimd.dma_start(y_d[:, t0:t0 + CHUNK], res[:])

    nc.compile()
    return nc


def _get_program():
    global _PROGRAM
    if _PROGRAM is None:
        _PROGRAM = _build_program()
    return _PROGRAM


# k order inside a parity-major dw row: evens then odds
_KORDER = list(range(0, K, 2)) + list(range(1, K, 2))


def _host_prep(spikes, conv_weight, conv_bias, dyn_weights, edge_send, edge_recv):
    import ml_dtypes
    bf = ml_dtypes.bfloat16

    spikes = np.asarray(spikes, dtype=np.float32)
    conv_weight = np.asarray(conv_weight, dtype=np.float32)
    conv_bias = np.asarray(conv_bias, dtype=np.float32)
    dyn_weights = np.asarray(dyn_weights, dtype=np.float32)
    edge_send = np.asarray(edge_send, dtype=np.int64)
    edge_recv = np.asarray(edge_recv, dtype=np.int64)

    x = np.ascontiguousarray(spikes[..., 0].transpose(0, 2, 1))  # [B, NVAR, T]
    dynb = dyn_weights.astype(bf)  # one bulk fp32->bf16 pass

    ssend = np.zeros((NVAR, E), bf)
    ssend[edge_send, np.arange(E)] = 1.0

    recvT = np.zeros((128, ETILES * NVAR), bf)
    for et in range(ETILES):
        rr = edge_recv[et * 128:(et + 1) * 128]
        recvT[np.arange(128), et * NVAR + rr] = 1.0

    w = conv_weight.copy()
    w[np.arange(NVAR), np.arange(NVAR), K - 1] = 0.0
    wt = np.ascontiguousarray(w.transpose(1, 2, 0)).reshape(NVAR, K * NVAR).astype(bf)

    biasv = conv_bias.reshape(NVAR, 1).astype(np.float32)

    in_maps = []
    for core in range(NC_COUNT):
        b, h = divmod(core, 2)
        tau0 = 0 if h == 0 else TAU - L  # 0 or 1023
        xpad = np.zeros((NVAR, W_XPAD), bf)
        lo = tau0 - (K - 2)  # first x column needed
        src_lo = max(lo, 0)
        xpad[:, src_lo - lo:W_XPAD - 1] = x[b, :, src_lo:tau0 + L + 1]
        a = dynb[:, b, tau0:tau0 + L, :]                 # [E, L, K]
        a = a.reshape(E, NCHUNK, CHUNK, K)               # [E, h2, tau, k]
        a = a.transpose(1, 0, 3, 2)[:, :, _KORDER, :]    # [h2, E, kpar, tau]
        dw = np.ascontiguousarray(a).reshape(NCHUNK * E, CHUNK * K)
        in_maps.append({
            "xpad": xpad,
            "dw": dw,
            "ssend": ssend,
            "wt": wt,
            "recvT": recvT,
            "biasv": biasv,
        })
    return in_maps


def _assemble(results):
    out = np.empty((B, TAU, NVAR, 1), np.float32)
    for core in range(NC_COUNT):
        b, h = divmod(core, 2)
        yT = results[core]["yT"]  # [NVAR, L]
        if h == 0:
            out[b, 0:L, :, 0] = yT.T
        else:
            out[b, L:TAU, :, 0] = yT[:, 1:L].T
    return out


def run_on_hw(in_maps, trace=False, **kwargs):
    from concourse.bass_utils import run_bass_kernel_spmd

    nc = _get_program()
    return run_bass_kernel_spmd(
        nc, in_maps, core_ids=list(range(NC_COUNT)), trace=trace, **kwargs
    )


def kernel(spikes, conv_weight, conv_bias, dyn_weights, edge_send, edge_recv):
    in_maps = _host_prep(
        spikes, conv_weight, conv_bias, dyn_weights, edge_send, edge_recv
    )
    res = run_on_hw(in_maps)
    return _assemble(res.results)
